# revision 40
# baseline (speedup 1.0000x reference)
"""CrossAttention kernel for Trainium2, 8 NeuronCores, batch-parallel.

Problem (hardcoded): B=16, S=4096, D=1024; K=77, DE=768; H=16, Dh=64.
  q = hs @ Wq; k = ehs @ Wk; v = ehs @ Wv   (per-head attention, softmax over 77)
  out = concat_heads(softmax(q k^T / 8) v) @ Wo + bo

Sharding: data-parallel over batch — core c gets batches [2c, 2c+1]. No collectives.

Per-core dataflow (Q path in float32r = full PE rate at free-dim>=256;
attention tail + out-proj in bf16, all accumulation fp32 in PSUM):
  - hs tiles are PE-transposed to hsT [D, s] so every GEMM contracts on partitions.
  - QT = Wq.T @ hsT (per 512-col s-tile, ACT evacuates), KT = Wk.T @ ehsT,
    V = ehs @ Wv (natural, interleaved with per-head ones columns).
  - scoresT[j,s] = KT_h.T @ QT_h (77x512 per head, f32r), exp on ACT -> bf16.
  - softmax sums for all 16 heads accumulate into ONE [16,512] PSUM bank via
    per-head selector matmuls; a single DVE reciprocal yields rec16; selector
    matmuls on the PE expand rec pairs to [128,512], ACT evacuates to SBUF.
  - o_ext = [V_h | 1]^T ones-column matmul per head; one DVE multiply per head
    (against the pair-broadcast reciprocals) normalizes into bf16 attT.
  - out[s,d] = attT.T @ Wo + bo (bias-add on DVE, contiguous DMA out).

Timing notes (measured via reps-differential, which cancels PJRT dispatch
overhead): one full 16-s-tile compute pass ~681 us of device time across the
8 cores (batch-parallel, so this is the whole-problem latency).
"""

import numpy as np

import concourse.bass as bass
import concourse.bacc as bacc
import concourse.mybir as mybir
from concourse.tile import TileContext
from concourse.bass_utils import run_bass_kernel_spmd
from concourse.masks import make_identity

# Problem constants
B, S, D = 16, 4096, 1024
KJ, DE = 77, 768
KJE = 78  # KJ padded to even for fp32r moving-operand ISA restriction
H, DH = 16, 64
INNER = H * DH  # 1024
NCORES = 8
BPC = B // NCORES  # batches per core = 2
ST = 512  # s-tile (columns of transposed activations)
NST = BPC * S // ST  # 16 s-tiles per core

F32 = mybir.dt.float32
F32R = mybir.dt.float32r
BF16 = mybir.dt.bfloat16
ACTF = mybir.ActivationFunctionType

_CACHE = {}

# walrus lowers Tile DMAs to PSEUDO_DMA_DIRECT2D, which cannot carry the
# two sync waits (data-dep + queue credit) Tile emits per DMA. Routing IO
# DMAs through DGE queues lifts that limit.
import concourse.bass_utils as _bu

if not getattr(_bu, "_dge_patched", False):
    _orig_run_command = _bu.run_command

    def _patched_run_command(argv, **kwargs):
        import os as _os

        if argv and "walrus_driver" in str(argv[0]):
            argv = list(argv)
            if not any("--dge-levels" in str(a) for a in argv):
                argv += ["--dge-levels=" + _os.environ.get("K_DGE_LEVELS", "io")]
            if _os.environ.get("K_STATIC_SP", "0") == "1":
                argv = [
                    "--assign-static-dmas-to-sp=true"
                    if str(a) == "--assign-static-dmas-to-sp=false"
                    else a
                    for a in argv
                ]
        return _orig_run_command(argv, **kwargs)

    _bu.run_command = _patched_run_command
    _bu._dge_patched = True


def _r(ap):
    return ap.bitcast(F32R)


def build_bass(reps=1, ps_big_bufs=3, ps_s_bufs=2, ps_o_bufs=2, fold_77=True,
               qt_bufs=8, att_bufs=12, hst_bufs=9, exp_bufs=17, big4k_bufs=7):
    nc = bacc.Bacc(use_seq_codegen=True)

    hs_d = nc.dram_tensor("hs", [BPC, S, D], F32, kind="ExternalInput")
    ehs_d = nc.dram_tensor("ehs", [BPC, KJ, DE], F32, kind="ExternalInput")
    wq_d = nc.dram_tensor("wq", [D, INNER], F32, kind="ExternalInput")
    wk_d = nc.dram_tensor("wk", [DE, INNER], F32, kind="ExternalInput")
    wv_d = nc.dram_tensor("wv", [DE, INNER], F32, kind="ExternalInput")
    wo_d = nc.dram_tensor("wo", [INNER, D], F32, kind="ExternalInput")
    bo_d = nc.dram_tensor("bo", [D], F32, kind="ExternalInput")
    out_d = nc.dram_tensor("out", [BPC, S, D], F32, kind="ExternalOutput")

    with TileContext(nc) as tc:
        with (
            tc.tile_pool(name="const", bufs=1) as constp,
            tc.tile_pool(name="wq", bufs=8) as wqp,
            tc.tile_pool(name="wo", bufs=8) as wop,
            tc.tile_pool(name="big4k", bufs=big4k_bufs) as big4k,
            tc.tile_pool(name="hst", bufs=hst_bufs) as hstp,
            tc.tile_pool(name="qt", bufs=qt_bufs) as qtp,
            tc.tile_pool(name="att", bufs=att_bufs) as attp,
            tc.tile_pool(name="expp", bufs=exp_bufs) as expp,
            tc.tile_pool(name="smalls", bufs=2) as smallp,
            tc.tile_pool(name="rbs", bufs=3) as rbsp,
            tc.tile_pool(name="ps_big", bufs=ps_big_bufs, space="PSUM") as ps_big,
            tc.tile_pool(name="ps_s", bufs=ps_s_bufs, space="PSUM") as ps_s,
            tc.tile_pool(name="ps_o", bufs=ps_o_bufs, space="PSUM") as ps_o,
            tc.tile_pool(name="ps_sum", bufs=1, space="PSUM") as ps_sum,
        ):
            from contextlib import ExitStack

            _es = ExitStack()
            _p77tag = "ps_big" if fold_77 else "p77"
            if fold_77:
                ps_77 = ps_big
            else:
                ps_77 = _es.enter_context(
                    tc.tile_pool(name="ps_77", bufs=1, space="PSUM")
                )
            # ---- constants / weights ----
            ident = constp.tile([128, 128], F32, tag="ident")
            make_identity(nc, ident)

            ones_col = constp.tile([128, 1], F32, tag="ones_col")
            nc.gpsimd.memset(ones_col, 1.0)
            zero_col = constp.tile([128, 1], F32, tag="zero_col")
            nc.gpsimd.memset(zero_col, 0.0)
            ones_bf = constp.tile([128, 1], BF16, tag="ones_bf")
            nc.gpsimd.memset(ones_bf, 1.0)

            # sums selector: sel[:, h*16+j] = 1 iff j == h; lhsT block for head
            # h accumulates that head's softmax sum into row h of psum16.
            sel = constp.tile([KJ, H * 16], BF16, tag="sel")
            nc.gpsimd.memset(sel, 0.0)
            for h in range(H):
                nc.vector.tensor_copy(sel[0:KJ, h * 16 + h:h * 16 + h + 1],
                                      ones_bf[0:KJ, 0:1])

            # rec-broadcast selector: selb[r, j*128+p] = 1 iff r == 2j + p//64
            selb = constp.tile([16, 8 * 128], BF16, tag="selb")
            nc.gpsimd.memset(selb, 0.0)
            nc.gpsimd.affine_select(
                out=selb,
                in_=selb,
                compare_op=mybir.AluOpType.not_equal,
                fill=1.0,
                base=0,
                pattern=[[-2, 8], [-1, 2], [0, 64]],
                channel_multiplier=1,
            )

            bo_sb = constp.tile([128, D], F32, tag="bo")
            nc.sync.dma_start(
                out=bo_sb, in_=bo_d[:].unsqueeze(0).to_broadcast((128, D))
            )

            wq_sb = []
            wo_sb = []
            for k in range(8):
                tq = big4k.tile([128, INNER], F32, tag="big4k", name=f"tq{k}")
                nc.sync.dma_start(out=tq, in_=wq_d[k * 128:(k + 1) * 128, :])
                wqk = wqp.tile([128, INNER], F32R, tag="wq", name=f"wq{k}")
                nc.vector.tensor_copy(wqk, tq)
                wq_sb.append(wqk)
                to = big4k.tile([128, D], F32, tag="big4k", name=f"to{k}")
                nc.sync.dma_start(out=to, in_=wo_d[k * 128:(k + 1) * 128, :])
                wok = wop.tile([128, D], BF16, tag="wo", name=f"wo{k}")
                nc.vector.tensor_copy(wok, to)
                wo_sb.append(wok)

            # ---- per-batch setup: ehsT, KT, V_ext ----
            kt_sb = [[None] * 8 for _ in range(BPC)]
            vext_sb = [None] * BPC
            for b in range(BPC):
                ehs_t = constp.tile([KJ, DE], F32, tag="ehs", bufs=2, name=f"ehs{b}")
                nc.sync.dma_start(out=ehs_t, in_=ehs_d[b, :, :])

                # ehsT tiles are padded to 78 cols: fp32r matmuls require an
                # even innermost moving-operand count (s3d3_mm_fp32r ISA
                # check). Col 77 is zeroed junk; its KT output is unread.
                ehsT = []
                for k in range(6):
                    pst = ps_77.tile([128, KJ], F32, tag=_p77tag, name=f"psT{b}_{k}")
                    nc.tensor.transpose(
                        pst[0:128, 0:KJ],
                        ehs_t[0:KJ, k * 128:(k + 1) * 128],
                        ident[0:KJ, 0:KJ],
                    )
                    et = constp.tile([128, KJE], F32R, tag=f"ehsT{k}", name=f"ehsT{b}_{k}")
                    nc.vector.tensor_copy(et[:, 0:KJ], pst[0:128, 0:KJ])
                    nc.vector.tensor_copy(
                        et[:, KJ:KJE], zero_col.bitcast(F32R)[0:128, 0:1]
                    )
                    ehsT.append(et)

                # Wk (shares big4k slots with Wv/hs_in/out tiles)
                wk_sb = []
                for k in range(6):
                    tk = big4k.tile([128, INNER], F32, tag="big4k", name=f"tk{b}_{k}")
                    nc.sync.dma_start(out=tk, in_=wk_d[k * 128:(k + 1) * 128, :])
                    wkk = big4k.tile(
                        [128, INNER], F32R, tag="big4k", name=f"wk{b}_{k}"
                    )
                    nc.vector.tensor_copy(wkk, tk)
                    wk_sb.append(wkk)
                # KT[m] = (Wk block m).T @ ehsT  -> [128 inner, 77] (+1 pad col)
                for m in range(8):
                    pkt = ps_77.tile([128, KJE], F32, tag=_p77tag, name=f"pkt{b}_{m}")
                    for k in range(6):
                        nc.tensor.matmul(
                            pkt[:, 0:KJE],
                            _r(wk_sb[k][:, m * 128:(m + 1) * 128]),
                            _r(ehsT[k][:, 0:KJE]),
                            start=(k == 0),
                            stop=(k == 5),
                        )
                    ktm = constp.tile([128, KJ], F32R, tag=f"kt{b}_{m}", name=f"kt{b}_{m}")
                    nc.vector.tensor_copy(ktm, pkt[:, 0:KJ])
                    kt_sb[b][m] = ktm

                # Wv then V natural layout [77, inner], interleaved with ones cols
                wv_sb = []
                for k in range(6):
                    tv = big4k.tile([128, INNER], F32, tag="big4k", name=f"tv{b}_{k}")
                    nc.sync.dma_start(out=tv, in_=wv_d[k * 128:(k + 1) * 128, :])
                    wvk = big4k.tile(
                        [128, INNER], F32R, tag="big4k", name=f"wv{b}_{k}"
                    )
                    nc.vector.tensor_copy(wvk, tv)
                    wv_sb.append(wvk)
                # fp32r memset is not a valid ISA op; fill the ones columns
                # (one per head, col h*65+64) from an F32 scratch via copy.
                vext = constp.tile([KJ, H * (DH + 1)], BF16, tag=f"vext{b}", name=f"vext{b}")
                for h in range(H):
                    nc.vector.tensor_copy(
                        vext[0:KJ, h * 65 + 64:h * 65 + 65],
                        ones_bf[0:KJ, 0:1],
                    )
                for n in range(2):
                    psv = ps_s.tile([KJ, 512], F32, tag="ps_s", name=f"psv{b}_{n}")
                    for k in range(6):
                        nc.tensor.matmul(
                            psv[0:KJ, :],
                            _r(ehsT[k][:, 0:KJ]),
                            _r(wv_sb[k][:, n * 512:(n + 1) * 512]),
                            start=(k == 0),
                            stop=(k == 5),
                        )
                    for j in range(8):
                        h = n * 8 + j
                        nc.vector.tensor_copy(
                            vext[0:KJ, h * 65:h * 65 + 64],
                            psv[0:KJ, j * 64:(j + 1) * 64],
                        )
                vext_sb[b] = vext

            # ---- main loop over s-tiles ----
            for t in range(reps * NST):
                tt = t % NST
                b = tt // (S // ST)
                s0 = (tt % (S // ST)) * ST

                # A: load + transpose hs -> hsT [D(8x128), ST]
                hs_in = []
                for r in range(4):
                    hin = big4k.tile([128, D], F32, tag="big4k", name=f"hsin{t}_{r}")
                    nc.sync.dma_start(
                        out=hin, in_=hs_d[b, s0 + r * 128:s0 + (r + 1) * 128, :]
                    )
                    hs_in.append(hin)
                hsT = []
                for k in range(8):
                    psx = ps_big.tile([128, ST], F32, tag="ps_big", name=f"psx{t}_{k}")
                    for r in range(4):
                        nc.tensor.transpose(
                            psx[:, r * 128:(r + 1) * 128],
                            hs_in[r][:, k * 128:(k + 1) * 128],
                            ident,
                        )
                    hk = hstp.tile([128, ST], F32R, tag="hst", name=f"hsT{t}_{k}")
                    nc.vector.tensor_copy(hk, psx)
                    hsT.append(hk)

                # B: QT = Wq.T @ hsT  -> 8 tiles [128, ST]
                qt = []
                for m in range(8):
                    psq = ps_big.tile([128, ST], F32, tag="ps_big", name=f"psq{t}_{m}")
                    for k in range(8):
                        nc.tensor.matmul(
                            psq,
                            _r(wq_sb[k][:, m * 128:(m + 1) * 128]),
                            _r(hsT[k]),
                            start=(k == 0),
                            stop=(k == 7),
                        )
                    qm = qtp.tile([128, ST], F32R, tag="qt", name=f"qt{t}_{m}")
                    nc.scalar.activation(qm, psq, mybir.ActivationFunctionType.Copy)
                    qt.append(qm)

                # C: attention per head -> attnT 8 tiles [128, ST]
                att = [
                    attp.tile([128, ST], BF16, tag="att", name=f"att{t}_{m}") for m in range(8)
                ]
                ex_t = [None] * H
                for h in range(H):
                    m, half = h // 2, h % 2
                    prow = slice(half * 64, half * 64 + 64)
                    pss = ps_s.tile([KJ, ST], F32, tag="ps_s", name=f"pss{t}_{h}")
                    nc.tensor.matmul(
                        pss[0:KJ, :],
                        _r(kt_sb[b][m][prow, 0:KJ]),
                        _r(qt[m][prow, :]),
                        start=True,
                        stop=True,
                    )
                    ex = expp.tile([KJ, ST], BF16, tag="exp", name=f"exp{t}_{h}")
                    nc.scalar.activation(
                        ex[0:KJ, :], pss[0:KJ, :], mybir.ActivationFunctionType.Exp
                    )
                    ex_t[h] = ex

                # batched softmax sums: 16 selector matmuls accumulate row h
                psum16 = ps_sum.tile([16, ST], F32, tag="ps_sum",
                                     name=f"psum16{t}")
                for h in range(H):
                    nc.tensor.matmul(
                        psum16[0:16, :],
                        sel[0:KJ, h * 16:(h + 1) * 16],
                        ex_t[h][0:KJ, :],
                        start=(h == 0),
                        stop=(h == H - 1),
                        skip_group_check=True,
                    )
                rec16 = smallp.tile([16, ST], F32, tag="rec", name=f"rec{t}")
                nc.vector.reciprocal(rec16[0:16, :], psum16[0:16, :])
                rec16b = smallp.tile([16, ST], BF16, tag="recb", name=f"recb{t}")
                nc.scalar.activation(rec16b[0:16, :], rec16[0:16, :], ACTF.Copy)

                # expand rec pairs to [128, ST] on the PE; ACT evacuates
                rbs_sb = [None] * 8
                for j in range(8):
                    rbp = ps_big.tile([128, ST], F32, tag="ps_big",
                                      name=f"rbp{t}_{j}")
                    nc.tensor.matmul(
                        rbp[0:128, :],
                        selb[0:16, j * 128:(j + 1) * 128],
                        rec16b[0:16, :],
                        start=True,
                        stop=True,
                    )
                    rb = rbsp.tile([128, ST], F32, tag="rbs", name=f"rbs{t}_{j}")
                    nc.scalar.activation(rb, rbp, ACTF.Copy)
                    rbs_sb[j] = rb

                for h in range(H):
                    m, half = h // 2, h % 2
                    prow = slice(half * 64, half * 64 + 64)
                    pso = ps_o.tile([65, ST], F32, tag="ps_o", name=f"pso{t}_{h}")
                    nc.tensor.matmul(
                        pso[0:65, :],
                        vext_sb[b][0:KJ, h * 65:(h + 1) * 65],
                        ex_t[h][0:KJ, :],
                        start=True,
                        stop=True,
                    )
                    nc.vector.tensor_mul(
                        att[m][prow, :], pso[0:64, :], rbs_sb[m][prow, :]
                    )

                # D: out = attnT.T @ Wo + bo  -> [128, 1024] x4, DMA out
                for r in range(4):
                    ot = big4k.tile([128, D], F32, tag="big4k", name=f"out{t}_{r}")
                    for n in range(2):
                        pso2 = ps_big.tile(
                            [128, 512], F32, tag="ps_big", name=f"pso2{t}_{r}_{n}"
                        )
                        for k in range(8):
                            nc.tensor.matmul(
                                pso2,
                                att[k][:, r * 128:(r + 1) * 128],
                                wo_sb[k][:, n * 512:(n + 1) * 512],
                                start=(k == 0),
                                stop=(k == 7),
                            )
                        nc.vector.tensor_add(
                            ot[:, n * 512:(n + 1) * 512],
                            pso2,
                            bo_sb[:, n * 512:(n + 1) * 512],
                        )
                    nc.sync.dma_start(
                        out=out_d[b, s0 + r * 128:s0 + (r + 1) * 128, :], in_=ot
                    )

            _es.close()

    nc.finalize()
    return nc


def kernel_jax(hidden_states, encoder_hidden_states, Wq, Wk, Wv, Wo, bo, **unused):
    """Batch-parallel cross-attention on 8 NeuronCores via the PJRT backend.

    Core c computes batches [2c, 2c+1]; outputs are concatenated on host.
    """
    import jax
    import jax.numpy as jnp

    if "jfn" not in _CACHE:

        def _f(hs, ehs, wq, wk, wv, wo, bo_):
            q = hs @ wq
            k = ehs @ wk
            v = ehs @ wv
            bpc, s, _ = hs.shape
            kj = ehs.shape[1]
            q = q.reshape(bpc, s, H, DH).transpose(0, 2, 1, 3)
            k = k.reshape(bpc, kj, H, DH).transpose(0, 2, 1, 3)
            v = v.reshape(bpc, kj, H, DH).transpose(0, 2, 1, 3)
            scores = jnp.einsum("bhsd,bhkd->bhsk", q, k) * (1.0 / np.sqrt(DH))
            probs = jax.nn.softmax(scores, axis=-1)
            out = jnp.einsum("bhsk,bhkd->bhsd", probs, v)
            out = out.transpose(0, 2, 1, 3).reshape(bpc, s, H * DH)
            return out @ wo + bo_

        _CACHE["jfn"] = jax.jit(_f)

    jfn = _CACHE["jfn"]
    devs = jax.devices()[:NCORES]
    hs = np.asarray(hidden_states, dtype=np.float32)
    ehs = np.asarray(encoder_hidden_states, dtype=np.float32)
    consts = [
        np.asarray(x, dtype=np.float32) for x in (Wq, Wk, Wv, Wo, bo)
    ]

    outs = []
    for c, d in enumerate(devs):
        args = [
            jax.device_put(np.ascontiguousarray(hs[c * BPC:(c + 1) * BPC]), d),
            jax.device_put(np.ascontiguousarray(ehs[c * BPC:(c + 1) * BPC]), d),
        ] + [jax.device_put(x, d) for x in consts]
        outs.append(jfn(*args))
    return np.concatenate([np.asarray(o) for o in outs], axis=0)


def kernel(hidden_states, encoder_hidden_states, Wq, Wk, Wv, Wo, bo, **unused):

    if "nc" not in _CACHE:
        _CACHE["nc"] = build_bass()
    nc = _CACHE["nc"]

    wq_scaled = (np.asarray(Wq, dtype=np.float32) * (1.0 / np.sqrt(DH))).astype(
        np.float32
    )
    wk = np.ascontiguousarray(np.asarray(Wk, dtype=np.float32))
    wv = np.ascontiguousarray(np.asarray(Wv, dtype=np.float32))
    wo = np.ascontiguousarray(np.asarray(Wo, dtype=np.float32))
    bo = np.ascontiguousarray(np.asarray(bo, dtype=np.float32))
    hs = np.asarray(hidden_states, dtype=np.float32)
    ehs = np.asarray(encoder_hidden_states, dtype=np.float32)

    in_maps = []
    for c in range(NCORES):
        in_maps.append(
            {
                "hs": np.ascontiguousarray(hs[c * BPC:(c + 1) * BPC]),
                "ehs": np.ascontiguousarray(ehs[c * BPC:(c + 1) * BPC]),
                "wq": wq_scaled,
                "wk": wk,
                "wv": wv,
                "wo": wo,
                "bo": bo,
            }
        )

    res = run_bass_kernel_spmd(nc, in_maps, list(range(NCORES)))
    outs = [res.results[c]["out"] for c in range(NCORES)]
    return np.concatenate(outs, axis=0)

